# revision 1
# baseline (speedup 1.0000x reference)
"""NeRF render kernel for 8 TRN2 NeuronCores (pure data parallel over rays).

Layout: activations [features(partition), points(free)]; tile = 1 sample x 512 rays,
grouped 4 samples/group (partition groups 32j). MLP matmuls in float32r (tf32-class);
sample 63 (the 1e10-delta sample, ReLU-sign-critical) runs in full fp32.
Fourier enc: magic-number range reduction on DVE + HW Sin spline (~1e-7 accurate).
Volume rendering: strict-lower-tri fp32 matmul for exclusive cumsum, then
INC=fl(EXCL+TAU), EXC2=fl(INC-TAU) to reproduce the reference's fp32 rounding at
the huge last-sample tau.
"""
import os
import numpy as np

NB = 10
ENC = 60
WIDTH = 256
S = 64
RPC = 512  # rays per core
N_CORES = 8
NEAR, FAR = 0.1, 4.0
MAGIC = float(1.5 * 2**23)
INV2PI = float(1.0 / (2 * np.pi))
TWO_PI_F32 = float(np.float32(2 * np.pi))
P2HI = 6.28125  # 2pi hi word, exact in 8 mantissa bits
P2LO = float(2 * np.pi - 6.28125)

LAST_EXEC_NS = None
_CACHE = {}


def _build_nc():
    import concourse.bacc as bacc
    import concourse.tile as tile
    from concourse import mybir

    dt = mybir.dt
    AF = mybir.ActivationFunctionType
    ALU = mybir.AluOpType
    f32 = dt.float32
    f32r = dt.float32r

    nc = bacc.Bacc("TRN2", target_bir_lowering=False, debug=False,
                   num_devices=N_CORES)

    def din(name, shape, dtype=f32):
        return nc.dram_tensor(name, shape, dtype, kind="ExternalInput")

    d_jit = din("jitter_t", [S, RPC])
    d_rp = din("ray_pos_t", [3, RPC])
    d_rd = din("ray_dir_t", [3, RPC])
    d_win_r = din("win_r", [ENC, WIDTH], f32r)
    d_win_32 = din("win_32", [ENC, WIDTH])
    d_whid_r = din("whid_r", [128, 7 * 2 * WIDTH], f32r)
    d_whid_32 = din("whid_32", [128, 7 * 2 * WIDTH])
    d_whd_r = din("whd_r", [128, 8], f32r)
    d_whd_32 = din("whd_32", [128, 8])
    d_brep_32 = din("brep_32", [128, ENC])
    d_ball = din("ball", [128, 16])
    d_bca = din("bca", [128, 1])
    d_bcb = din("bcb", [S, 1])
    d_bcd = din("bcd", [S, 1])
    d_mbias = din("mbias", [ENC, 1])
    d_b2 = din("b2", [ENC, 1])
    d_iota = din("iota", [S, 1])
    d_tris = din("tris", [S, S])
    d_onesb = din("onesb", [128, 2])
    d_big = din("big", [1, RPC])
    d_out = nc.dram_tensor("out", [4, RPC], f32, kind="ExternalOutput")

    DEBUG = bool(os.environ.get("KERNEL_DEBUG"))
    dbg = {}
    if DEBUG:
        for nm, shp in (("dbg_dd", [S, RPC]), ("dbg_jpos", [128, RPC]),
                        ("dbg_xb", [ENC, RPC]), ("dbg_r", [ENC, RPC]),
                        ("dbg_enc", [ENC, RPC]), ("dbg_x0", [128, RPC]),
                        ("dbg_x7b", [128, RPC]), ("dbg_stg", [4, RPC]),
                        ("dbg_den", [S, RPC]), ("dbg_rgba", [128, RPC]),
                        ("dbg_tau", [S, RPC]), ("dbg_trans", [S, RPC]),
                        ("dbg_w", [S, RPC])):
            dbg[nm] = nc.dram_tensor(nm, shp, f32, kind="ExternalOutput")

    with tile.TileContext(nc) as tc:
        with (
            tc.tile_pool(name="static", bufs=1) as sp,
            tc.tile_pool(name="act", bufs=3) as ap,
            tc.tile_pool(name="comp", bufs=1) as cp,
            tc.tile_pool(name="ps_xb", bufs=2, space="PSUM") as pxb,
            tc.tile_pool(name="ps_l", bufs=4, space="PSUM") as pl,
            tc.tile_pool(name="ps_h", bufs=2, space="PSUM") as ph,
        ):
            def load(dram, shape, dtype, tag):
                t = sp.tile(shape, dtype, tag=tag)
                nc.sync.dma_start(t[:], dram[:])
                return t

            win_r = load(d_win_r, [ENC, WIDTH], f32r, "win_r")
            win_32 = load(d_win_32, [ENC, WIDTH], f32, "win_32")
            whid_r = load(d_whid_r, [128, 7 * 2 * WIDTH], f32r, "whid_r")
            whid_32 = load(d_whid_32, [128, 7 * 2 * WIDTH], f32, "whid_32")
            whd_r = load(d_whd_r, [128, 8], f32r, "whd_r")
            whd_32 = load(d_whd_32, [128, 8], f32, "whd_32")
            brep_32 = load(d_brep_32, [128, ENC], f32, "brep_32")
            ball = load(d_ball, [128, 16], f32, "ball")
            bca = load(d_bca, [128, 1], f32, "bca")
            bcb = load(d_bcb, [S, 1], f32, "bcb")
            bcd = load(d_bcd, [S, 1], f32, "bcd")
            mbias = load(d_mbias, [ENC, 1], f32, "mbias")
            b2 = load(d_b2, [ENC, 1], f32, "b2")
            iota = load(d_iota, [S, 1], f32, "iota")
            tris = load(d_tris, [S, S], f32, "tris")
            onesb = load(d_onesb, [128, 2], f32, "onesb")
            jt = load(d_jit, [S, RPC], f32, "jt")

            rp128 = sp.tile([128, RPC], f32, tag="rp128")
            rd128 = sp.tile([128, RPC], f32, tag="rd128")
            for j in range(4):
                nc.sync.dma_start(rp128[32 * j:32 * j + 3, :], d_rp[:, :])
                nc.sync.dma_start(rd128[32 * j:32 * j + 3, :], d_rd[:, :])

            # depths, with exactly the reference's fp32 op sequence:
            # depths = 0.1 + (3.9 * (idx + jitter)) / 64
            ddtmp = sp.tile([S, RPC], f32, tag="ddtmp")
            nc.vector.tensor_scalar(ddtmp[:], jt[:], iota[:], 3.9, ALU.add, ALU.mult)
            dd = sp.tile([S, RPC], f32, tag="dd")
            nc.vector.tensor_scalar(dd[:], ddtmp[:], float(1.0 / 64), 0.1, ALU.mult, ALU.add)

            # composite accumulation buffers
            rgba = cp.tile([128, RPC], f32, tag="rgba")   # rows 0-63 rgb0, 64-127 rgb1
            rgbb = cp.tile([128, RPC], f32, tag="rgbb")   # rows 0-63 rgb2, 64-127 depths
            den = cp.tile([S, RPC], f32, tag="den")

            def mlp_tile(enc_t, acc, wmats, s):
                """8-layer MLP + heads for one 512-ray tile; enc_t [60, 512].

                Head outputs (raw pre-activation z) are scattered into the
                composite buffers at sample row s; activations applied later.
                """
                win, whid, whd = wmats
                xdt = f32 if acc else f32r
                xtag = "x32" if acc else "xr"
                xa = xb_ = None
                for layer in range(8):
                    pa = pl.tile([128, RPC], f32, tag="lp")
                    pb = pl.tile([128, RPC], f32, tag="lp")
                    if layer == 0:
                        nc.tensor.matmul(pa[:], win[:, 0:128], enc_t[:], start=True, stop=True)
                        nc.tensor.matmul(pb[:], win[:, 128:256], enc_t[:], start=True, stop=True)
                    else:
                        for mc, ps in ((0, pa), (1, pb)):
                            for kc, xt in ((0, xa), (1, xb_)):
                                base = ((layer - 1) * 2 + kc) * WIDTH + mc * 128
                                nc.tensor.matmul(ps[:], whid[:, base:base + 128], xt[:],
                                                 start=(kc == 0), stop=(kc == 1))
                    na = ap.tile([128, RPC], xdt, tag=xtag + "a")
                    nb_ = ap.tile([128, RPC], xdt, tag=xtag + "b")
                    nc.vector.tensor_scalar(na[:], pa[:], ball[:, 2 * layer:2 * layer + 1],
                                            0.0, ALU.add, ALU.max)
                    nc.scalar.activation(nb_[:], pb[:], AF.Relu,
                                         bias=ball[:, 2 * layer + 1:2 * layer + 2])
                    xa, xb_ = na, nb_
                    if DEBUG and s == 0 and layer == 0:
                        nc.sync.dma_start(dbg["dbg_x0"][:], xa[:].bitcast(f32))
                hp = ph.tile([4, RPC], f32, tag="hp")
                for kc, xt in ((0, xa), (1, xb_)):
                    nc.tensor.matmul(hp[:], whd[:, kc * 4:kc * 4 + 4], xt[:],
                                     start=(kc == 0), stop=(kc == 1))
                stg = ap.tile([4, RPC], f32, tag="stg")
                nc.vector.tensor_copy(stg[:], hp[:])
                if DEBUG and s == 0:
                    nc.sync.dma_start(dbg["dbg_x7b"][:], xb_[:].bitcast(f32))
                    nc.sync.dma_start(dbg["dbg_stg"][:], stg[:])
                nc.sync.dma_start(rgba[s:s + 1, :], stg[0:1, :])
                nc.sync.dma_start(rgba[S + s:S + s + 1, :], stg[1:2, :])
                nc.sync.dma_start(rgbb[s:s + 1, :], stg[2:3, :])
                nc.sync.dma_start(den[s:s + 1, :], stg[3:4, :])

            for g in range(16):
                s0 = 4 * g
                acc_grp = (g == 15)
                dd4 = ap.tile([128, RPC], f32, tag="dd4")
                for i in range(3):
                    nc.sync.dma_start(dd4[i::32, :], dd[s0:s0 + 4, :])
                jtmp = ap.tile([128, RPC], f32, tag="jtmp")
                nc.vector.tensor_mul(jtmp[:], dd4[:], rd128[:])
                jpos32 = ap.tile([128, RPC], f32, tag="jpos32")
                nc.vector.tensor_add(jpos32[:], jtmp[:], rp128[:])

                for j in range(4):
                    s = s0 + j
                    tile63 = (s == 63)
                    xbp = pxb.tile([ENC, RPC], f32, tag="xb")
                    nc.tensor.matmul(xbp[:], brep_32[32 * j:32 * j + 3, :],
                                     jpos32[32 * j:32 * j + 3, :], start=True,
                                     stop=True, tile_position=(32 * j, 0))
                    # range reduction: r = xb + b2 - n*2pi,  n = round(xb/2pi + 0.25*iscos)
                    rt = ap.tile([ENC, RPC], f32, tag="red_t")
                    nc.vector.tensor_scalar(rt[:], xbp[:], INV2PI, mbias[:], ALU.mult, ALU.add)
                    rn = ap.tile([ENC, RPC], f32, tag="red_n")
                    nc.vector.tensor_scalar(rn[:], rt[:], MAGIC, MAGIC, ALU.add, ALU.subtract)
                    ru = ap.tile([ENC, RPC], f32, tag="red_u")
                    if acc_grp:
                        nc.vector.tensor_scalar(ru[:], rn[:], -P2HI, None, ALU.mult)
                    else:
                        nc.vector.tensor_scalar(ru[:], rn[:], -TWO_PI_F32, None, ALU.mult)
                    rr = ap.tile([ENC, RPC], f32, tag="red_r")
                    nc.vector.scalar_tensor_tensor(rr[:], ru[:], b2[:], xbp[:], ALU.add, ALU.add)
                    if acc_grp:
                        ru2 = ap.tile([ENC, RPC], f32, tag="red_u2")
                        nc.vector.tensor_scalar(ru2[:], rn[:], P2LO, None, ALU.mult)
                        nc.vector.tensor_sub(rr[:], rr[:], ru2[:])
                    enc_t = ap.tile([ENC, RPC], f32 if tile63 else f32r,
                                    tag="enc32" if tile63 else "encr")
                    nc.scalar.activation(enc_t[:], rr[:], AF.Sin)
                    if DEBUG and s == 0:
                        xbc = ap.tile([ENC, RPC], f32, tag="dbgxbc")
                        nc.vector.tensor_copy(xbc[:], xbp[:])
                        nc.sync.dma_start(dbg["dbg_xb"][:], xbc[:])
                        nc.sync.dma_start(dbg["dbg_r"][:], rr[:])
                        nc.sync.dma_start(dbg["dbg_enc"][:], enc_t[:].bitcast(f32))
                        nc.sync.dma_start(dbg["dbg_jpos"][:], jpos32[:])
                    if tile63:
                        mlp_tile(enc_t, True, (win_32, whid_32, whd_32), s)
                    else:
                        mlp_tile(enc_t, False, (win_r, whid_r, whd_r), s)

            # ---- head activations (batched, wide lanes) ----
            # rgb = 0.5 + 0.5*tanh(0.5*z + 0.5*b_rgb); den = relu(z + b_den)
            tmpa = cp.tile([128, RPC], f32, tag="tmpa")
            nc.scalar.activation(tmpa[:], rgba[:], AF.Tanh, bias=bca[:], scale=0.5)
            nc.vector.tensor_scalar(rgba[:], tmpa[:], 0.5, 0.5, ALU.mult, ALU.add)
            tmpb = cp.tile([S, RPC], f32, tag="tmpb")
            nc.scalar.activation(tmpb[:], rgbb[0:S, :], AF.Tanh, bias=bcb[:], scale=0.5)
            nc.vector.tensor_scalar(rgbb[0:S, :], tmpb[:], 0.5, 0.5, ALU.mult, ALU.add)
            denr = cp.tile([S, RPC], f32, tag="denr")
            nc.vector.tensor_scalar(denr[:], den[:], bcd[:], 0.0, ALU.add, ALU.max)

            # ---- volume rendering composite ----
            ddsh = cp.tile([S, RPC], f32, tag="ddsh")
            nc.sync.dma_start(ddsh[0:63, :], dd[1:64, :])
            nc.sync.dma_start(ddsh[63:64, :], d_big[:])
            delt = cp.tile([S, RPC], f32, tag="delt")
            nc.vector.tensor_sub(delt[:], ddsh[:], dd[:])
            tau = cp.tile([S, RPC], f32, tag="tau")
            nc.vector.tensor_mul(tau[:], denr[:], delt[:])
            exclp = ph.tile([128, RPC], f32, tag="hp")
            nc.tensor.matmul(exclp[0:S, :], tris[:], tau[:], start=True, stop=True)
            inc = cp.tile([S, RPC], f32, tag="inc")
            nc.vector.tensor_add(inc[:], exclp[0:S, :], tau[:])
            exc2 = cp.tile([S, RPC], f32, tag="exc2")
            nc.vector.tensor_sub(exc2[:], inc[:], tau[:])
            trans = cp.tile([S, RPC], f32, tag="trans")
            nc.scalar.activation(trans[:], exc2[:], AF.Exp, scale=-1.0)
            ee = cp.tile([S, RPC], f32, tag="ee")
            nc.scalar.activation(ee[:], tau[:], AF.Exp, scale=-1.0)
            alpha = cp.tile([S, RPC], f32, tag="alpha")
            nc.vector.tensor_scalar(alpha[:], ee[:], -1.0, 1.0, ALU.mult, ALU.add)
            wt = cp.tile([S, RPC], f32, tag="wt")
            nc.vector.tensor_mul(wt[:], alpha[:], trans[:])
            if DEBUG:
                nc.sync.dma_start(dbg["dbg_dd"][:], dd[:])
                nc.sync.dma_start(dbg["dbg_den"][:], den[:])
                nc.sync.dma_start(dbg["dbg_rgba"][:], rgba[:])
                nc.sync.dma_start(dbg["dbg_tau"][:], tau[:])
                nc.sync.dma_start(dbg["dbg_trans"][:], trans[:])
                nc.sync.dma_start(dbg["dbg_w"][:], wt[:])
            w2 = cp.tile([128, RPC], f32, tag="w2")
            nc.sync.dma_start(w2[0:S, :], wt[:])
            nc.sync.dma_start(w2[S:128, :], wt[:])
            nc.sync.dma_start(rgbb[S:128, :], dd[:])
            wa = cp.tile([128, RPC], f32, tag="wa")
            nc.vector.tensor_mul(wa[:], w2[:], rgba[:])
            wb = cp.tile([128, RPC], f32, tag="wb")
            nc.vector.tensor_mul(wb[:], w2[:], rgbb[:])
            redp = ph.tile([128, RPC], f32, tag="hp")
            nc.tensor.matmul(redp[0:2, :], onesb[:], wa[:], start=True, stop=True)
            nc.tensor.matmul(redp[32:34, :], onesb[:], wb[:], start=True, stop=True)
            outsb = cp.tile([S, RPC], f32, tag="outsb")
            nc.vector.tensor_copy(outsb[0:2, :], redp[0:2, :])
            nc.vector.tensor_copy(outsb[32:34, :], redp[32:34, :])
            nc.sync.dma_start(d_out[0:2, :], outsb[0:2, :])
            nc.sync.dma_start(d_out[2:4, :], outsb[32:34, :])

    nc.compile()
    return nc


def _prep(inputs):
    jt = np.ascontiguousarray(np.asarray(inputs["jitter"], np.float32).T)
    rpt = np.ascontiguousarray(np.asarray(inputs["ray_pos"], np.float32).T)
    rdt = np.ascontiguousarray(np.asarray(inputs["ray_dir"], np.float32).T)
    w_in = np.asarray(inputs["w_in"], np.float32)
    perm = np.empty(ENC, np.int64)
    for r in range(ENC):
        base = 0 if r < 30 else 10
        rr = r % 30
        perm[r] = (rr // 10) * 20 + base + (rr % 10)
    win_p = np.ascontiguousarray(w_in[perm])
    w_hid = np.asarray(inputs["w_hid"], np.float32)
    whid_cat = np.empty((128, 7 * 2 * WIDTH), np.float32)
    for l in range(7):
        for kc in range(2):
            whid_cat[:, (l * 2 + kc) * WIDTH:(l * 2 + kc + 1) * WIDTH] = \
                w_hid[l, kc * 128:(kc + 1) * 128, :]
    whd = np.concatenate([np.asarray(inputs["w_rgb"], np.float32),
                          np.asarray(inputs["w_den"], np.float32)], axis=1)
    whd_cat = np.empty((128, 8), np.float32)
    whd_cat[:, 0:4] = whd[0:128]
    whd_cat[:, 4:8] = whd[128:256]
    b_in = np.asarray(inputs["b_in"], np.float32)
    b_hid = np.asarray(inputs["b_hid"], np.float32)
    ball = np.zeros((128, 16), np.float32)
    for l in range(8):
        b = b_in if l == 0 else b_hid[l - 1]
        ball[:, 2 * l] = b[0:128]
        ball[:, 2 * l + 1] = b[128:256]
    b_rgb = np.asarray(inputs["b_rgb"], np.float32)
    b_den = np.asarray(inputs["b_den"], np.float32)
    bca = np.zeros((128, 1), np.float32)
    bca[0:S] = 0.5 * b_rgb[0]
    bca[S:128] = 0.5 * b_rgb[1]
    bcb = np.full((S, 1), 0.5 * b_rgb[2], np.float32)
    bcd = np.full((S, 1), b_den[0], np.float32)
    brep = np.zeros((128, ENC), np.float32)
    for r in range(ENC):
        rr = r % 30
        i, k = rr // 10, rr % 10
        for j in range(4):
            brep[32 * j + i, r] = float(2.0 ** k)
    mbias = np.zeros((ENC, 1), np.float32)
    mbias[30:] = 0.25
    b2v = np.zeros((ENC, 1), np.float32)
    b2v[30:] = np.float32(np.pi / 2)
    iota = np.arange(S, dtype=np.float32).reshape(S, 1)
    tris = (np.arange(S)[:, None] < np.arange(S)[None, :]).astype(np.float32)
    onesb = np.zeros((128, 2), np.float32)
    onesb[:64, 0] = 1.0
    onesb[64:, 1] = 1.0
    big = np.full((1, RPC), 1e10, np.float32)
    common = dict(win_r=win_p, win_32=win_p, whid_r=whid_cat, whid_32=whid_cat,
                  whd_r=whd_cat, whd_32=whd_cat, brep_32=brep,
                  ball=ball, bca=bca, bcb=bcb, bcd=bcd, mbias=mbias, b2=b2v, iota=iota,
                  tris=tris, onesb=onesb, big=big)
    in_maps = []
    for c in range(N_CORES):
        sl = slice(c * RPC, (c + 1) * RPC)
        m = dict(common)
        m["jitter_t"] = np.ascontiguousarray(jt[:, sl])
        m["ray_pos_t"] = np.ascontiguousarray(rpt[:, sl])
        m["ray_dir_t"] = np.ascontiguousarray(rdt[:, sl])
        in_maps.append(m)
    return in_maps


def kernel(**inputs):
    global LAST_EXEC_NS
    from concourse.bass_utils import run_bass_kernel_spmd
    if "nc" not in _CACHE:
        _CACHE["nc"] = _build_nc()
    nc = _CACHE["nc"]
    in_maps = _prep(inputs)
    trace = bool(os.environ.get("KERNEL_TRACE"))
    res = run_bass_kernel_spmd(nc, in_maps, core_ids=list(range(N_CORES)),
                               trace=trace)
    LAST_EXEC_NS = getattr(res, "exec_time_ns", None)
    _CACHE["last_results"] = res.results
    out = np.empty((N_CORES * RPC, 4), np.float32)
    for c in range(N_CORES):
        out[c * RPC:(c + 1) * RPC] = res.results[c]["out"].T
    return out

